# revision 7
# baseline (speedup 1.0000x reference)
"""BitLinear (BitNet b1.58) Trainium2 kernel, 8-core data-parallel.

Reference computation (fp32):
    scale  = 127 / clip(max|x| over d_in, 1e-5)          (per token)
    xq     = clip(round(x*scale), -128, 127) / scale     (per-token int8 quant-dequant)
    s      = clip(mean|W|, 1e-5)
    wq     = clip(round(W/s), -1, 1) * s                 (ternary quant)
    out    = xq @ wq.T

Kernel strategy (per core, tokens sharded 4096/core, weight replicated):
    q  = round(x*scale)  are integers in [-127,127]  -> exact in bf16
    t  = clip(round(W/s),-1,1) in {-1,0,1}           -> exact in bf16
    q @ t.T accumulated in fp32 PSUM is EXACT integer arithmetic, then
    out = psum * (absmax * s / 127) per token.

    Rounding uses the fp32 magic-number trick  round(v) = (v + 1.5*2^23) - 1.5*2^23.
    x is rounded BEFORE the PE transpose (ACT does x*scl+MAGIC, DVE subtracts
    MAGIC with bf16 output), so the transposes run at bf16 rate (1 cyc/row
    instead of 2).  Weight phase: w DMAs are split across both HWDGE queues and
    dispatched first; per-arrival PE transposes + DVE column sums hide under the
    DMA; ternarize works in the magic domain (ACT: w/s+MAGIC, DVE: clip at
    MAGIC+-1, ACT/DVE: -MAGIC -> bf16) per k-tile so tT[k] completes
    incrementally, while the first two output tiles' matmul chains interleave
    k-major to keep the PE busy during the ternarize window.
"""

import numpy as np

import concourse.bass as bass
import concourse.mybir as mybir
from concourse import tile, masks
from concourse.bass_utils import run_bass_kernel_spmd

F32 = mybir.dt.float32
BF16 = mybir.dt.bfloat16

N_CORES = 8
B, S, D_IN, D_OUT = 4, 8192, 1024, 1024
TOKENS = B * S                     # 32768
TOK_PER_CORE = TOKENS // N_CORES   # 4096
TILES = TOK_PER_CORE // 128        # 32
KT = D_IN // 128                   # 8 contraction k-tiles
OB = D_OUT // 128                  # 8 output row blocks of W

EPS = 1e-5
QMAX = 127.0
MAGIC = 12582912.0                 # 1.5 * 2**23 -> RNE integer rounding


def _split_multiwaits(nc):
    """walrus here encodes at most ONE sem wait per instruction; Tile's tail
    drain (and occasionally other insts) carry several.  Split extras into
    single-wait NOPs on the same engine, preserving order."""
    for f in nc.m.functions:
        for bb in f.blocks:
            insts = list(bb.instructions)
            if not any(
                i.sync_info and len(i.sync_info.on_wait) > 1 for i in insts
            ):
                continue
            new = []
            for ins in insts:
                si = ins.sync_info
                if si and len(si.on_wait) > 1:
                    waits = list(si.on_wait)
                    for j, w in enumerate(waits[:-1]):
                        nop = mybir.InstNoOp(
                            name=f"{ins.name}_wsp{j}", ins=[], outs=[]
                        )
                        nop.engine = ins.engine
                        nop.sync_info = mybir.SyncInfo(on_wait=[w], on_update=[])
                        new.append(nop)
                    ins.sync_info = mybir.SyncInfo(
                        on_wait=[waits[-1]], on_update=list(si.on_update)
                    )
                new.append(ins)
            bb.instructions = new


def build_program():
    nc = bass.Bass(trn_type="TRN2")
    x_d = nc.dram_tensor("x", [TOK_PER_CORE, D_IN], F32, kind="ExternalInput")
    w_d = nc.dram_tensor("weight", [D_OUT, D_IN], F32, kind="ExternalInput")
    o_d = nc.dram_tensor("out", [TOK_PER_CORE, D_OUT], F32, kind="ExternalOutput")

    Copy = mybir.ActivationFunctionType.Copy
    AX = mybir.AxisListType.X
    op = mybir.AluOpType

    with tile.TileContext(nc) as tc:
        from contextlib import ExitStack

        with ExitStack() as ctx:
            singles = ctx.enter_context(tc.tile_pool(name="singles", bufs=1))

            ident_f32 = singles.tile([128, 128], F32)
            ident_bf = singles.tile([128, 128], BF16)
            ones_col = singles.tile([128, 1], F32)
            ones_row = singles.tile([1, 128], F32)
            bc2 = singles.tile([128, 2], F32)      # [s, 1/s] broadcast
            s127_bc = singles.tile([128, 1], F32)  # s/127 broadcast

            tT = [
                singles.tile([128, D_OUT], BF16, name=f"tT{k}", tag=f"tT{k}")
                for k in range(KT)
            ]

            xpool = ctx.enter_context(tc.tile_pool(name="xpool", bufs=8))
            xmpool = ctx.enter_context(tc.tile_pool(name="xmpool", bufs=4))
            qrpool = ctx.enter_context(tc.tile_pool(name="qrpool", bufs=4))
            qtpool = ctx.enter_context(tc.tile_pool(name="qtpool", bufs=6))
            outpool = ctx.enter_context(tc.tile_pool(name="outpool", bufs=4))
            smpool = ctx.enter_context(tc.tile_pool(name="smpool", bufs=10))
            psq = ctx.enter_context(tc.tile_pool(name="psq", bufs=2, space="PSUM"))
            pso = ctx.enter_context(tc.tile_pool(name="pso", bufs=4, space="PSUM"))

            live = {}

            def a_disp(n, eng):
                x_t = xpool.tile([128, D_IN], F32, tag="x", name="x_t")
                eng.dma_start(x_t[:], x_d[n * 128:(n + 1) * 128, :])
                live[("x", n)] = x_t

            def a_stats(n):
                x_t = live[("x", n)]
                am = smpool.tile([128, 1], F32, tag="am", name="am")
                nc.vector.tensor_reduce(
                    am[:], x_t[:], axis=AX, op=op.max, apply_absolute_value=True
                )
                amc = smpool.tile([128, 1], F32, tag="amc", name="amc")
                nc.vector.tensor_scalar_max(amc[:], am[:], EPS)
                ram = smpool.tile([128, 1], F32, tag="ram", name="ram")
                nc.vector.reciprocal(ram[:], amc[:])
                scl = smpool.tile([128, 1], F32, tag="scl", name="scl")
                nc.vector.tensor_scalar(scl[:], ram[:], QMAX, None, op0=op.mult)
                live[("amc", n)] = amc
                live[("scl", n)] = scl

            def a_round(n, dve=False):
                # xm = x*scl + MAGIC (fp32), qr = xm - MAGIC -> bf16 (exact ints)
                x_t = live.pop(("x", n))
                scl = live.pop(("scl", n))
                xm = xmpool.tile([128, D_IN], F32, tag="xm", name="xm")
                qr = qrpool.tile([128, D_IN], BF16, tag="qr", name="qr")
                if dve:
                    # keep ACT free: both passes on DVE
                    nc.vector.tensor_scalar(
                        xm[:], x_t[:], scl[:], MAGIC, op0=op.mult, op1=op.add
                    )
                    nc.vector.tensor_scalar(qr[:], xm[:], -MAGIC, None, op0=op.add)
                else:
                    nc.scalar.activation(xm[:], x_t[:], Copy, bias=MAGIC, scale=scl[:])
                    nc.vector.tensor_scalar(qr[:], xm[:], -MAGIC, None, op0=op.add)
                live[("qr", n)] = qr

            def a_xpose(n):
                qr = live.pop(("qr", n))
                ps_q = psq.tile([128, D_IN], BF16, tag="ps_q", name="ps_q")
                for k in range(KT):
                    nc.tensor.transpose(
                        ps_q[:, k * 128:(k + 1) * 128],
                        qr[:, k * 128:(k + 1) * 128],
                        ident_bf[:],
                    )
                qT = qtpool.tile([128, D_IN], BF16, tag="qT", name="qT")
                nc.scalar.activation(qT[:], ps_q[:], Copy)
                live[("qT", n)] = qT

            def a_quant(n):
                a_stats(n)
                a_round(n)
                a_xpose(n)

            def b_mm(ps_pair, qT, k):
                nc.tensor.matmul(
                    ps_pair[0][:], qT[:, k * 128:(k + 1) * 128],
                    tT[k][:, 0:512], start=(k == 0), stop=(k == KT - 1),
                )
                nc.tensor.matmul(
                    ps_pair[1][:], qT[:, k * 128:(k + 1) * 128],
                    tT[k][:, 512:1024], start=(k == 0), stop=(k == KT - 1),
                )

            def b_coef(n):
                amc = live.pop(("amc", n))
                coef = smpool.tile([128, 1], F32, tag="coef", name="coef")
                nc.vector.tensor_scalar(coef[:], amc[:], s127_bc[:], None, op0=op.mult)
                return coef

            def b_drain(n, ps_pair):
                coef = b_coef(n)
                out_sb = outpool.tile([128, D_OUT], F32, tag="osb", name="out_sb")
                nc.scalar.activation(
                    out_sb[:, 0:512], ps_pair[0][:], Copy, scale=coef[:]
                )
                nc.sync.dma_start(
                    o_d[n * 128:(n + 1) * 128, 0:512], out_sb[:, 0:512]
                )
                nc.vector.tensor_scalar(
                    out_sb[:, 512:1024], ps_pair[1][:], coef[:], None, op0=op.mult
                )
                nc.sync.dma_start(
                    o_d[n * 128:(n + 1) * 128, 512:1024], out_sb[:, 512:1024]
                )

            def b(n):
                # oh-major: oh0 chain completes first so its drain + store
                # overlap the oh1 chain
                qT = live.pop(("qT", n))
                coef = b_coef(n)
                out_sb = outpool.tile([128, D_OUT], F32, tag="osb", name="out_sb")
                for oh in range(2):
                    ps_o = pso.tile([128, 512], F32, tag="ps_oh", name="ps_o")
                    for k in range(KT):
                        nc.tensor.matmul(
                            ps_o[:], qT[:, k * 128:(k + 1) * 128],
                            tT[k][:, oh * 512:(oh + 1) * 512],
                            start=(k == 0), stop=(k == KT - 1),
                        )
                    sl = out_sb[:, oh * 512:(oh + 1) * 512]
                    if oh == 0:
                        nc.scalar.activation(sl, ps_o[:], Copy, scale=coef[:])
                    else:
                        nc.vector.tensor_scalar(sl, ps_o[:], coef[:], None, op0=op.mult)
                    nc.sync.dma_start(
                        o_d[n * 128:(n + 1) * 128, oh * 512:(oh + 1) * 512], sl
                    )

            # ---------------- weight phase ---------------------------------
            with tc.tile_pool(name="wpool", bufs=1) as wpool, \
                 tc.tile_pool(name="wtmp", bufs=3) as wtmp:
                w_t = [
                    wpool.tile([128, D_IN], F32, name=f"w{ob}", tag=f"w{ob}")
                    for ob in range(OB)
                ]
                # w first on both queues, then x tiles behind
                for ob in range(4):
                    nc.sync.dma_start(w_t[ob][:], w_d[ob * 128:(ob + 1) * 128, :])
                for ob in range(4, OB):
                    nc.scalar.dma_start(w_t[ob][:], w_d[ob * 128:(ob + 1) * 128, :])
                a_disp(0, nc.sync)
                a_disp(1, nc.scalar)
                a_disp(2, nc.sync)
                a_disp(3, nc.sync)

                masks.make_identity(nc, ident_f32[:])
                masks.make_identity(nc, ident_bf[:])
                nc.vector.memset(ones_col[:], 1.0)
                nc.vector.memset(ones_row[:], 1.0)

                # raw w transposed, fp32: wT_all[:, k, o] = w[o, k*128 + p]
                wT_all = wpool.tile([128, KT, D_OUT], F32, name="wT_all")
                colsum = wpool.tile([128, OB], F32, name="colsum")
                for ob in range(OB):
                    nc.vector.tensor_reduce(
                        colsum[:, ob:ob + 1], w_t[ob][:], axis=AX, op=op.add,
                        apply_absolute_value=True,
                    )
                    po_w = psq.tile([128, KT, 128], F32, tag="ps_q", name="po_w")
                    for k in range(KT):
                        nc.tensor.transpose(
                            po_w[:, k, :],
                            w_t[ob][:, k * 128:(k + 1) * 128],
                            ident_f32[:],
                        )
                    nc.scalar.activation(
                        wT_all[:, :, ob * 128:(ob + 1) * 128], po_w[:], Copy
                    )

                # global scale s = clip(mean|W|, EPS); bc2 = [s, 1/s] broadcast
                colsum2 = wpool.tile([128, 1], F32, name="colsum2")
                nc.vector.tensor_reduce(colsum2[:], colsum[:], axis=AX, op=op.add)
                ps_st = psq.tile([128, D_IN], F32, tag="ps_q", name="ps_st")
                nc.tensor.matmul(ps_st[0:1, 0:1], ones_col[:], colsum2[:])
                pair = wpool.tile([1, 2], F32, name="pair")
                nc.scalar.activation(
                    pair[:, 0:1], ps_st[0:1, 0:1], Copy, scale=1.0 / (D_OUT * D_IN)
                )
                nc.vector.tensor_scalar_max(pair[:, 0:1], pair[:, 0:1], EPS)
                nc.vector.reciprocal(pair[:, 1:2], pair[:, 0:1])
                ps_st2 = psq.tile([128, D_IN], F32, tag="ps_q", name="ps_st2")
                nc.tensor.matmul(ps_st2[:, 0:2], ones_row[:], pair[:])
                nc.scalar.copy(bc2[:], ps_st2[:, 0:2])
                nc.vector.tensor_scalar(
                    s127_bc[:], bc2[:, 0:1], 1.0 / QMAX, None, op0=op.mult
                )

                # ternarize per k (magic domain) + interleaved b0/b1 chains +
                # x0..x3 prep, all mixed so no engine serializes the window
                ch = [
                    [
                        pso.tile([128, 512], F32, tag="ps_oh", name=f"ch{i}_{oh}")
                        for oh in range(2)
                    ]
                    for i in range(2)
                ]
                ACT_UNMAGIC = {3, 6}
                GP_UNMAGIC = {1, 4}
                for it in range(KT + 3):
                    if it < KT:
                        k = it
                        y0 = wtmp.tile([128, D_OUT], F32, name="y0", tag="y0")
                        nc.scalar.activation(
                            y0[:], wT_all[:, k, :], Copy, bias=MAGIC,
                            scale=bc2[:, 1:2],
                        )
                        y1 = wtmp.tile([128, D_OUT], F32, name="y1", tag="y1")
                        nc.vector.tensor_scalar(
                            y1[:], y0[:], MAGIC + 1.0, MAGIC - 1.0,
                            op0=op.min, op1=op.max,
                        )
                        if k in ACT_UNMAGIC:
                            nc.scalar.activation(tT[k][:], y1[:], Copy, bias=-MAGIC)
                        elif k in GP_UNMAGIC:
                            nc.gpsimd.tensor_scalar(
                                tT[k][:], y1[:], -MAGIC, None, op0=op.add
                            )
                        else:
                            nc.vector.tensor_scalar(
                                tT[k][:], y1[:], -MAGIC, None, op0=op.add
                            )
                    if it == 0:
                        a_stats(0)
                        a_round(0, dve=True)
                    elif it == 1:
                        a_xpose(0)
                        a_stats(1)
                        a_round(1, dve=True)
                    elif it == 2:
                        a_xpose(1)
                    if 2 <= it < 2 + KT:
                        b_mm(ch[0], live[("qT", 0)], it - 2)
                    if 3 <= it < 3 + KT:
                        b_mm(ch[1], live[("qT", 1)], it - 3)
                    if it == 3:
                        a_disp(4, nc.sync)
                    elif it == 4:
                        a_stats(2)
                        a_round(2)
                    elif it == 5:
                        a_xpose(2)
                        a_disp(5, nc.sync)
                    elif it == 6:
                        a_stats(3)
                        a_round(3)
                    elif it == 7:
                        a_xpose(3)
                        a_disp(6, nc.sync)
                    elif it == 8:
                        a_disp(7, nc.sync)
                live.pop(("qT", 0))
                live.pop(("qT", 1))

            b_drain(0, ch[0])
            b_drain(1, ch[1])

            # ---------------- steady state ---------------------------------
            for n in range(2, TILES):
                m = n + 2
                if 4 <= m < TILES:
                    a_stats(m)
                    a_round(m)
                b(n)
                if 4 <= m < TILES:
                    a_xpose(m)
                if n + 6 < TILES:
                    a_disp(n + 6, nc.sync)

    _split_multiwaits(nc)
    return nc


_NC_CACHE = None


def _get_nc():
    global _NC_CACHE
    if _NC_CACHE is None:
        _NC_CACHE = build_program()
    return _NC_CACHE


def kernel(x: np.ndarray, weight: np.ndarray, trace: bool = False):
    assert x.shape == (B, S, D_IN) and weight.shape == (D_OUT, D_IN)
    nc = _get_nc()
    xf = np.ascontiguousarray(x.reshape(TOKENS, D_IN), dtype=np.float32)
    w = np.ascontiguousarray(weight, dtype=np.float32)
    in_maps = [
        {
            "x": xf[c * TOK_PER_CORE:(c + 1) * TOK_PER_CORE],
            "weight": w,
        }
        for c in range(N_CORES)
    ]
    res = run_bass_kernel_spmd(nc, in_maps, core_ids=list(range(N_CORES)), trace=trace)
    kernel.last_results = res
    out = np.concatenate([res.results[c]["out"] for c in range(N_CORES)], axis=0)
    return out.reshape(B, S, D_OUT)


kernel.last_results = None


# revision 8
# speedup vs baseline: 1.0774x; 1.0774x over previous
"""BitLinear (BitNet b1.58) Trainium2 kernel, 8-core data-parallel.

Reference computation (fp32):
    scale  = 127 / clip(max|x| over d_in, 1e-5)          (per token)
    xq     = clip(round(x*scale), -128, 127) / scale     (per-token int8 quant-dequant)
    s      = clip(mean|W|, 1e-5)
    wq     = clip(round(W/s), -1, 1) * s                 (ternary quant)
    out    = xq @ wq.T

Kernel strategy (per core, tokens sharded 4096/core, weight replicated):
    q  = round(x*scale)  are integers in [-127,127]  -> exact in bf16
    t  = clip(round(W/s),-1,1) in {-1,0,1}           -> exact in bf16
    q @ t.T accumulated in fp32 PSUM is EXACT integer arithmetic, then
    out = psum * (absmax * s / 127) per token.

    Rounding uses the fp32 magic-number trick  round(v) = (v + 1.5*2^23) - 1.5*2^23.
    x is rounded BEFORE the PE transpose (ACT does x*scl+MAGIC, DVE subtracts
    MAGIC with bf16 output), so the transposes run at bf16 rate (1 cyc/row
    instead of 2).  Weight phase: w DMAs are split across both HWDGE queues and
    dispatched first; per-arrival PE transposes + DVE column sums hide under the
    DMA; ternarize works in the magic domain (ACT: w/s+MAGIC, DVE: clip at
    MAGIC+-1, ACT/DVE: -MAGIC -> bf16) per k-tile so tT[k] completes
    incrementally, while the first two output tiles' matmul chains interleave
    k-major to keep the PE busy during the ternarize window.
"""

import numpy as np

import concourse.bass as bass
import concourse.mybir as mybir
from concourse import tile, masks
from concourse.bass_utils import run_bass_kernel_spmd

F32 = mybir.dt.float32
BF16 = mybir.dt.bfloat16

N_CORES = 8
B, S, D_IN, D_OUT = 4, 8192, 1024, 1024
TOKENS = B * S                     # 32768
TOK_PER_CORE = TOKENS // N_CORES   # 4096
TILES = TOK_PER_CORE // 128        # 32
KT = D_IN // 128                   # 8 contraction k-tiles
OB = D_OUT // 128                  # 8 output row blocks of W

EPS = 1e-5
QMAX = 127.0
MAGIC = 12582912.0                 # 1.5 * 2**23 -> RNE integer rounding


def _split_multiwaits(nc):
    """walrus here encodes at most ONE sem wait per instruction; Tile's tail
    drain (and occasionally other insts) carry several.  Split extras into
    single-wait NOPs on the same engine, preserving order."""
    for f in nc.m.functions:
        for bb in f.blocks:
            insts = list(bb.instructions)
            if not any(
                i.sync_info and len(i.sync_info.on_wait) > 1 for i in insts
            ):
                continue
            new = []
            for ins in insts:
                si = ins.sync_info
                if si and len(si.on_wait) > 1:
                    waits = list(si.on_wait)
                    for j, w in enumerate(waits[:-1]):
                        nop = mybir.InstNoOp(
                            name=f"{ins.name}_wsp{j}", ins=[], outs=[]
                        )
                        nop.engine = ins.engine
                        nop.sync_info = mybir.SyncInfo(on_wait=[w], on_update=[])
                        new.append(nop)
                    ins.sync_info = mybir.SyncInfo(
                        on_wait=[waits[-1]], on_update=list(si.on_update)
                    )
                new.append(ins)
            bb.instructions = new


def build_program():
    nc = bass.Bass(trn_type="TRN2")
    x_d = nc.dram_tensor("x", [TOK_PER_CORE, D_IN], F32, kind="ExternalInput")
    w_d = nc.dram_tensor("weight", [D_OUT, D_IN], F32, kind="ExternalInput")
    o_d = nc.dram_tensor("out", [TOK_PER_CORE, D_OUT], F32, kind="ExternalOutput")

    Copy = mybir.ActivationFunctionType.Copy
    AX = mybir.AxisListType.X
    op = mybir.AluOpType

    with tile.TileContext(nc) as tc:
        from contextlib import ExitStack

        with ExitStack() as ctx:
            singles = ctx.enter_context(tc.tile_pool(name="singles", bufs=1))

            ident_f32 = singles.tile([128, 128], F32)
            ident_bf = singles.tile([128, 128], BF16)
            ones_col = singles.tile([128, 1], F32)
            ones_row = singles.tile([1, 128], F32)
            bc2 = singles.tile([128, 2], F32)      # [s, 1/s] broadcast
            s127_bc = singles.tile([128, 1], F32)  # s/127 broadcast

            tT = [
                singles.tile([128, D_OUT], BF16, name=f"tT{k}", tag=f"tT{k}")
                for k in range(KT)
            ]

            xpool = ctx.enter_context(tc.tile_pool(name="xpool", bufs=8))
            xmpool = ctx.enter_context(tc.tile_pool(name="xmpool", bufs=4))
            qrpool = ctx.enter_context(tc.tile_pool(name="qrpool", bufs=4))
            qtpool = ctx.enter_context(tc.tile_pool(name="qtpool", bufs=6))
            outpool = ctx.enter_context(tc.tile_pool(name="outpool", bufs=4))
            smpool = ctx.enter_context(tc.tile_pool(name="smpool", bufs=10))
            psq = ctx.enter_context(tc.tile_pool(name="psq", bufs=2, space="PSUM"))
            pso = ctx.enter_context(tc.tile_pool(name="pso", bufs=4, space="PSUM"))

            live = {}

            def a_disp(n, eng):
                x_t = xpool.tile([128, D_IN], F32, tag="x", name="x_t")
                eng.dma_start(x_t[:], x_d[n * 128:(n + 1) * 128, :])
                live[("x", n)] = x_t

            def a_stats(n):
                x_t = live[("x", n)]
                am = smpool.tile([128, 1], F32, tag="am", name="am")
                nc.vector.tensor_reduce(
                    am[:], x_t[:], axis=AX, op=op.max, apply_absolute_value=True
                )
                amc = smpool.tile([128, 1], F32, tag="amc", name="amc")
                nc.vector.tensor_scalar_max(amc[:], am[:], EPS)
                ram = smpool.tile([128, 1], F32, tag="ram", name="ram")
                nc.vector.reciprocal(ram[:], amc[:])
                scl = smpool.tile([128, 1], F32, tag="scl", name="scl")
                nc.vector.tensor_scalar(scl[:], ram[:], QMAX, None, op0=op.mult)
                live[("amc", n)] = amc
                live[("scl", n)] = scl

            def a_round(n, dve=False):
                # xm = x*scl + MAGIC (fp32), qr = xm - MAGIC -> bf16 (exact ints)
                x_t = live.pop(("x", n))
                scl = live.pop(("scl", n))
                xm = xmpool.tile([128, D_IN], F32, tag="xm", name="xm")
                qr = qrpool.tile([128, D_IN], BF16, tag="qr", name="qr")
                if dve:
                    # keep ACT free: both passes on DVE
                    nc.vector.tensor_scalar(
                        xm[:], x_t[:], scl[:], MAGIC, op0=op.mult, op1=op.add
                    )
                    nc.vector.tensor_scalar(qr[:], xm[:], -MAGIC, None, op0=op.add)
                else:
                    nc.scalar.activation(xm[:], x_t[:], Copy, bias=MAGIC, scale=scl[:])
                    nc.vector.tensor_scalar(qr[:], xm[:], -MAGIC, None, op0=op.add)
                live[("qr", n)] = qr

            def a_xpose(n):
                qr = live.pop(("qr", n))
                ps_q = psq.tile([128, D_IN], BF16, tag="ps_q", name="ps_q")
                for k in range(KT):
                    nc.tensor.transpose(
                        ps_q[:, k * 128:(k + 1) * 128],
                        qr[:, k * 128:(k + 1) * 128],
                        ident_bf[:],
                    )
                qT = qtpool.tile([128, D_IN], BF16, tag="qT", name="qT")
                nc.scalar.activation(qT[:], ps_q[:], Copy)
                live[("qT", n)] = qT

            def a_quant(n):
                a_stats(n)
                a_round(n)
                a_xpose(n)

            def b_mm(ps_pair, qT, k):
                nc.tensor.matmul(
                    ps_pair[0][:], qT[:, k * 128:(k + 1) * 128],
                    tT[k][:, 0:512], start=(k == 0), stop=(k == KT - 1),
                )
                nc.tensor.matmul(
                    ps_pair[1][:], qT[:, k * 128:(k + 1) * 128],
                    tT[k][:, 512:1024], start=(k == 0), stop=(k == KT - 1),
                )

            def b_coef(n):
                amc = live.pop(("amc", n))
                coef = smpool.tile([128, 1], F32, tag="coef", name="coef")
                nc.vector.tensor_scalar(coef[:], amc[:], s127_bc[:], None, op0=op.mult)
                return coef

            def b_drain(n, ps_pair):
                coef = b_coef(n)
                out_sb = outpool.tile([128, D_OUT], F32, tag="osb", name="out_sb")
                nc.scalar.activation(
                    out_sb[:, 0:512], ps_pair[0][:], Copy, scale=coef[:]
                )
                nc.sync.dma_start(
                    o_d[n * 128:(n + 1) * 128, 0:512], out_sb[:, 0:512]
                )
                nc.vector.tensor_scalar(
                    out_sb[:, 512:1024], ps_pair[1][:], coef[:], None, op0=op.mult
                )
                nc.sync.dma_start(
                    o_d[n * 128:(n + 1) * 128, 512:1024], out_sb[:, 512:1024]
                )

            def b(n):
                # oh-major: oh0 chain completes first so its drain + store
                # overlap the oh1 chain
                qT = live.pop(("qT", n))
                coef = b_coef(n)
                out_sb = outpool.tile([128, D_OUT], F32, tag="osb", name="out_sb")
                for oh in range(2):
                    ps_o = pso.tile([128, 512], F32, tag="ps_oh", name="ps_o")
                    for k in range(KT):
                        nc.tensor.matmul(
                            ps_o[:], qT[:, k * 128:(k + 1) * 128],
                            tT[k][:, oh * 512:(oh + 1) * 512],
                            start=(k == 0), stop=(k == KT - 1),
                        )
                    sl = out_sb[:, oh * 512:(oh + 1) * 512]
                    if oh == 0:
                        nc.scalar.activation(sl, ps_o[:], Copy, scale=coef[:])
                    else:
                        nc.vector.tensor_scalar(sl, ps_o[:], coef[:], None, op0=op.mult)
                    nc.sync.dma_start(
                        o_d[n * 128:(n + 1) * 128, oh * 512:(oh + 1) * 512], sl
                    )

            # ---------------- weight phase ---------------------------------
            with tc.tile_pool(name="wpool", bufs=1) as wpool, \
                 tc.tile_pool(name="wtmp", bufs=3) as wtmp:
                w_t = [
                    wpool.tile([128, D_IN], F32, name=f"w{ob}", tag=f"w{ob}")
                    for ob in range(OB)
                ]
                # w first on both queues, then x tiles behind
                for ob in range(4):
                    nc.sync.dma_start(w_t[ob][:], w_d[ob * 128:(ob + 1) * 128, :])
                for ob in range(4, OB):
                    nc.scalar.dma_start(w_t[ob][:], w_d[ob * 128:(ob + 1) * 128, :])
                a_disp(0, nc.sync)
                a_disp(1, nc.scalar)
                a_disp(2, nc.sync)
                a_disp(3, nc.sync)

                masks.make_identity(nc, ident_f32[:])
                masks.make_identity(nc, ident_bf[:])
                nc.vector.memset(ones_col[:], 1.0)
                nc.vector.memset(ones_row[:], 1.0)

                # raw w transposed, fp32: wT_all[:, k, o] = w[o, k*128 + p]
                wT_all = wpool.tile([128, KT, D_OUT], F32, name="wT_all")
                colsum = wpool.tile([128, OB], F32, name="colsum")
                for ob in range(OB):
                    nc.vector.tensor_reduce(
                        colsum[:, ob:ob + 1], w_t[ob][:], axis=AX, op=op.add,
                        apply_absolute_value=True,
                    )
                    po_w = psq.tile([128, KT, 128], F32, tag="ps_q", name="po_w")
                    for k in range(KT):
                        nc.tensor.transpose(
                            po_w[:, k, :],
                            w_t[ob][:, k * 128:(k + 1) * 128],
                            ident_f32[:],
                        )
                    nc.scalar.activation(
                        wT_all[:, :, ob * 128:(ob + 1) * 128], po_w[:], Copy
                    )

                # global scale s = clip(mean|W|, EPS); bc2 = [s, 1/s] broadcast
                colsum2 = wpool.tile([128, 1], F32, name="colsum2")
                nc.vector.tensor_reduce(colsum2[:], colsum[:], axis=AX, op=op.add)
                ps_st = psq.tile([128, D_IN], F32, tag="ps_q", name="ps_st")
                nc.tensor.matmul(ps_st[0:1, 0:1], ones_col[:], colsum2[:])
                pair = wpool.tile([1, 2], F32, name="pair")
                nc.scalar.activation(
                    pair[:, 0:1], ps_st[0:1, 0:1], Copy, scale=1.0 / (D_OUT * D_IN)
                )
                nc.vector.tensor_scalar_max(pair[:, 0:1], pair[:, 0:1], EPS)
                nc.vector.reciprocal(pair[:, 1:2], pair[:, 0:1])
                ps_st2 = psq.tile([128, D_IN], F32, tag="ps_q", name="ps_st2")
                nc.tensor.matmul(ps_st2[:, 0:2], ones_row[:], pair[:])
                nc.scalar.copy(bc2[:], ps_st2[:, 0:2])
                nc.vector.tensor_scalar(
                    s127_bc[:], bc2[:, 0:1], 1.0 / QMAX, None, op0=op.mult
                )

                # ternarize per k (magic domain) + interleaved b0/b1 chains +
                # x0..x3 prep, all mixed so no engine serializes the window
                ch = [
                    [
                        pso.tile([128, 512], F32, tag="ps_oh", name=f"ch{i}_{oh}")
                        for oh in range(2)
                    ]
                    for i in range(2)
                ]
                ACT_UNMAGIC = {3, 6}
                GP_UNMAGIC = set()   # gpsimd measured ~15us per [128,1024] pass — useless
                for it in range(KT + 3):
                    if it < KT:
                        k = it
                        y0 = wtmp.tile([128, D_OUT], F32, name="y0", tag="y0")
                        nc.scalar.activation(
                            y0[:], wT_all[:, k, :], Copy, bias=MAGIC,
                            scale=bc2[:, 1:2],
                        )
                        y1 = wtmp.tile([128, D_OUT], F32, name="y1", tag="y1")
                        nc.vector.tensor_scalar(
                            y1[:], y0[:], MAGIC + 1.0, MAGIC - 1.0,
                            op0=op.min, op1=op.max,
                        )
                        if k in ACT_UNMAGIC:
                            nc.scalar.activation(tT[k][:], y1[:], Copy, bias=-MAGIC)
                        elif k in GP_UNMAGIC:
                            nc.gpsimd.tensor_scalar(
                                tT[k][:], y1[:], -MAGIC, None, op0=op.add
                            )
                        else:
                            nc.vector.tensor_scalar(
                                tT[k][:], y1[:], -MAGIC, None, op0=op.add
                            )
                    if it == 0:
                        a_stats(0)
                        a_round(0, dve=True)
                    elif it == 1:
                        a_xpose(0)
                        a_stats(1)
                        a_round(1, dve=True)
                    elif it == 2:
                        a_xpose(1)
                    if 2 <= it < 2 + KT:
                        b_mm(ch[0], live[("qT", 0)], it - 2)
                    if 3 <= it < 3 + KT:
                        b_mm(ch[1], live[("qT", 1)], it - 3)
                    if it == 3:
                        a_disp(4, nc.sync)
                    elif it == 4:
                        a_stats(2)
                        a_round(2)
                    elif it == 5:
                        a_xpose(2)
                        a_disp(5, nc.sync)
                    elif it == 6:
                        a_stats(3)
                        a_round(3)
                    elif it == 7:
                        a_xpose(3)
                        a_disp(6, nc.sync)
                    elif it == 8:
                        a_disp(7, nc.sync)
                live.pop(("qT", 0))
                live.pop(("qT", 1))

            b_drain(0, ch[0])
            b_drain(1, ch[1])

            # ---------------- steady state ---------------------------------
            for n in range(2, TILES):
                m = n + 2
                if 4 <= m < TILES:
                    a_stats(m)
                    a_round(m)
                b(n)
                if 4 <= m < TILES:
                    a_xpose(m)
                if n + 6 < TILES:
                    a_disp(n + 6, nc.sync)

    _split_multiwaits(nc)
    return nc


_NC_CACHE = None


def _get_nc():
    global _NC_CACHE
    if _NC_CACHE is None:
        _NC_CACHE = build_program()
    return _NC_CACHE


def kernel(x: np.ndarray, weight: np.ndarray, trace: bool = False):
    assert x.shape == (B, S, D_IN) and weight.shape == (D_OUT, D_IN)
    nc = _get_nc()
    xf = np.ascontiguousarray(x.reshape(TOKENS, D_IN), dtype=np.float32)
    w = np.ascontiguousarray(weight, dtype=np.float32)
    in_maps = [
        {
            "x": xf[c * TOK_PER_CORE:(c + 1) * TOK_PER_CORE],
            "weight": w,
        }
        for c in range(N_CORES)
    ]
    res = run_bass_kernel_spmd(nc, in_maps, core_ids=list(range(N_CORES)), trace=trace)
    kernel.last_results = res
    out = np.concatenate([res.results[c]["out"] for c in range(N_CORES)], axis=0)
    return out.reshape(B, S, D_OUT)


kernel.last_results = None


# revision 14
# speedup vs baseline: 1.1725x; 1.0883x over previous
"""BitLinear (BitNet b1.58) Trainium2 kernel, 8-core data-parallel.

Reference computation (fp32):
    scale  = 127 / clip(max|x| over d_in, 1e-5)          (per token)
    xq     = clip(round(x*scale), -128, 127) / scale     (per-token int8 quant-dequant)
    s      = clip(mean|W|, 1e-5)
    wq     = clip(round(W/s), -1, 1) * s                 (ternary quant)
    out    = xq @ wq.T

Kernel strategy (per core, tokens sharded 4096/core, weight replicated):
    q  = round(x*scale)  are integers in [-127,127]  -> exact in bf16
    t  = clip(round(W/s),-1,1) in {-1,0,1}           -> exact in bf16
    q @ t.T accumulated in fp32 PSUM is EXACT integer arithmetic, then
    out = psum * (absmax * s / 127) per token.

    Rounding uses the fp32 magic-number trick  round(v) = (v + 1.5*2^23) - 1.5*2^23.
    x is rounded BEFORE the PE transpose (ACT does x*scl+MAGIC, DVE subtracts
    MAGIC with bf16 output), so the transposes run at bf16 rate (1 cyc/row
    instead of 2).  Weight phase: w DMAs are split across both HWDGE queues and
    dispatched first; per-arrival PE transposes + DVE column sums hide under the
    DMA; ternarize works in the magic domain (ACT: w/s+MAGIC, DVE: clip at
    MAGIC+-1, ACT/DVE: -MAGIC -> bf16) per k-tile so tT[k] completes
    incrementally, while the first two output tiles' matmul chains interleave
    k-major to keep the PE busy during the ternarize window.
"""

import numpy as np

import concourse.bass as bass
import concourse.mybir as mybir
from concourse import tile, masks
from concourse.bass_utils import run_bass_kernel_spmd

F32 = mybir.dt.float32
BF16 = mybir.dt.bfloat16

N_CORES = 8
B, S, D_IN, D_OUT = 4, 8192, 1024, 1024
TOKENS = B * S                     # 32768
TOK_PER_CORE = TOKENS // N_CORES   # 4096
TILES = TOK_PER_CORE // 128        # 32
KT = D_IN // 128                   # 8 contraction k-tiles
OB = D_OUT // 128                  # 8 output row blocks of W

EPS = 1e-5
QMAX = 127.0
MAGIC = 12582912.0                 # 1.5 * 2**23 -> RNE integer rounding


def _split_multiwaits(nc):
    """walrus here encodes at most ONE sem wait per instruction; Tile's tail
    drain (and occasionally other insts) carry several.  Split extras into
    single-wait NOPs on the same engine, preserving order."""
    for f in nc.m.functions:
        for bb in f.blocks:
            insts = list(bb.instructions)
            if not any(
                i.sync_info and len(i.sync_info.on_wait) > 1 for i in insts
            ):
                continue
            new = []
            for ins in insts:
                si = ins.sync_info
                if si and len(si.on_wait) > 1:
                    waits = list(si.on_wait)
                    for j, w in enumerate(waits[:-1]):
                        nop = mybir.InstNoOp(
                            name=f"{ins.name}_wsp{j}", ins=[], outs=[]
                        )
                        nop.engine = ins.engine
                        nop.sync_info = mybir.SyncInfo(on_wait=[w], on_update=[])
                        new.append(nop)
                    ins.sync_info = mybir.SyncInfo(
                        on_wait=[waits[-1]], on_update=list(si.on_update)
                    )
                new.append(ins)
            bb.instructions = new


def build_program():
    nc = bass.Bass(trn_type="TRN2")
    x_d = nc.dram_tensor("x", [TOK_PER_CORE, D_IN], F32, kind="ExternalInput")
    w_d = nc.dram_tensor("weight", [D_OUT, D_IN], F32, kind="ExternalInput")
    o_d = nc.dram_tensor("out", [TOK_PER_CORE, D_OUT], F32, kind="ExternalOutput")

    Copy = mybir.ActivationFunctionType.Copy
    AX = mybir.AxisListType.X
    op = mybir.AluOpType

    with tile.TileContext(nc) as tc:
        from contextlib import ExitStack

        with ExitStack() as ctx:
            singles = ctx.enter_context(tc.tile_pool(name="singles", bufs=1))

            ident_f32 = singles.tile([128, 128], F32)
            ident_bf = singles.tile([128, 128], BF16)
            ones_col = singles.tile([128, 1], F32)
            ones_row = singles.tile([1, 128], F32)
            bc2 = singles.tile([128, 2], F32)      # [s, 1/s] broadcast
            s127_bc = singles.tile([128, 1], F32)  # s/127 broadcast

            tT = [
                singles.tile([128, D_OUT], BF16, name=f"tT{k}", tag=f"tT{k}")
                for k in range(KT)
            ]

            xpool = ctx.enter_context(tc.tile_pool(name="xpool", bufs=8))
            xmpool = ctx.enter_context(tc.tile_pool(name="xmpool", bufs=4))
            qrpool = ctx.enter_context(tc.tile_pool(name="qrpool", bufs=4))
            qtpool = ctx.enter_context(tc.tile_pool(name="qtpool", bufs=6))
            outpool = ctx.enter_context(tc.tile_pool(name="outpool", bufs=4))
            smpool = ctx.enter_context(tc.tile_pool(name="smpool", bufs=10))
            # single unified PSUM pool: 8 slots x 2KB/partition (one bank each)
            psu = ctx.enter_context(tc.tile_pool(name="psu", bufs=8, space="PSUM"))

            live = {}

            def a_disp(n, eng):
                x_t = xpool.tile([128, D_IN], F32, tag="x", name="x_t")
                eng.dma_start(x_t[:], x_d[n * 128:(n + 1) * 128, :])
                live[("x", n)] = x_t

            def a_stats(n):
                x_t = live[("x", n)]
                am = smpool.tile([128, 1], F32, tag="am", name="am")
                nc.vector.tensor_reduce(
                    am[:], x_t[:], axis=AX, op=op.max, apply_absolute_value=True
                )
                amc = smpool.tile([128, 1], F32, tag="amc", name="amc")
                nc.vector.tensor_scalar_max(amc[:], am[:], EPS)
                ram = smpool.tile([128, 1], F32, tag="ram", name="ram")
                nc.vector.reciprocal(ram[:], amc[:])
                scl = smpool.tile([128, 1], F32, tag="scl", name="scl")
                nc.vector.tensor_scalar(scl[:], ram[:], QMAX, None, op0=op.mult)
                live[("amc", n)] = amc
                live[("scl", n)] = scl

            def a_round(n, dve=False):
                # xm = x*scl + MAGIC (fp32), qr = xm - MAGIC -> bf16 (exact ints)
                x_t = live.pop(("x", n))
                scl = live.pop(("scl", n))
                xm = xmpool.tile([128, D_IN], F32, tag="xm", name="xm")
                qr = qrpool.tile([128, D_IN], BF16, tag="qr", name="qr")
                if dve:
                    # keep ACT free: both passes on DVE
                    nc.vector.tensor_scalar(
                        xm[:], x_t[:], scl[:], MAGIC, op0=op.mult, op1=op.add
                    )
                    nc.vector.tensor_scalar(qr[:], xm[:], -MAGIC, None, op0=op.add)
                else:
                    nc.scalar.activation(xm[:], x_t[:], Copy, bias=MAGIC, scale=scl[:])
                    nc.vector.tensor_scalar(qr[:], xm[:], -MAGIC, None, op0=op.add)
                live[("qr", n)] = qr

            def a_xpose(n):
                qr = live.pop(("qr", n))
                ps_q = psu.tile([128, D_IN], BF16, tag="ps", name="ps_q")
                for k in range(KT):
                    nc.tensor.transpose(
                        ps_q[:, k * 128:(k + 1) * 128],
                        qr[:, k * 128:(k + 1) * 128],
                        ident_bf[:],
                    )
                qT = qtpool.tile([128, D_IN], BF16, tag="qT", name="qT")
                nc.scalar.activation(qT[:], ps_q[:], Copy)
                live[("qT", n)] = qT

            def a_quant(n):
                a_stats(n)
                a_round(n)
                a_xpose(n)

            def b_mm(ps_pair, qT, k):
                nc.tensor.matmul(
                    ps_pair[0][:], qT[:, k * 128:(k + 1) * 128],
                    tT[k][:, 0:512], start=(k == 0), stop=(k == KT - 1),
                )
                nc.tensor.matmul(
                    ps_pair[1][:], qT[:, k * 128:(k + 1) * 128],
                    tT[k][:, 512:1024], start=(k == 0), stop=(k == KT - 1),
                )

            def b_coef(n):
                amc = live.pop(("amc", n))
                coef = smpool.tile([128, 1], F32, tag="coef", name="coef")
                nc.vector.tensor_scalar(coef[:], amc[:], s127_bc[:], None, op0=op.mult)
                return coef

            def b_drain(n, ps_pair):
                coef = b_coef(n)
                out_sb = outpool.tile([128, D_OUT], F32, tag="osb", name="out_sb")
                nc.scalar.activation(
                    out_sb[:, 0:512], ps_pair[0][:], Copy, scale=coef[:]
                )
                nc.sync.dma_start(
                    o_d[n * 128:(n + 1) * 128, 0:512], out_sb[:, 0:512]
                )
                nc.vector.tensor_scalar(
                    out_sb[:, 512:1024], ps_pair[1][:], coef[:], None, op0=op.mult
                )
                nc.sync.dma_start(
                    o_d[n * 128:(n + 1) * 128, 512:1024], out_sb[:, 512:1024]
                )

            def b(n):
                # oh-major: oh0 chain completes first so its drain + store
                # overlap the oh1 chain
                qT = live.pop(("qT", n))
                coef = b_coef(n)
                out_sb = outpool.tile([128, D_OUT], F32, tag="osb", name="out_sb")
                for oh in range(2):
                    ps_o = psu.tile([128, 512], F32, tag="ps", name="ps_o")
                    for k in range(KT):
                        nc.tensor.matmul(
                            ps_o[:], qT[:, k * 128:(k + 1) * 128],
                            tT[k][:, oh * 512:(oh + 1) * 512],
                            start=(k == 0), stop=(k == KT - 1),
                        )
                    sl = out_sb[:, oh * 512:(oh + 1) * 512]
                    if oh == 0:
                        nc.scalar.activation(sl, ps_o[:], Copy, scale=coef[:])
                    else:
                        nc.vector.tensor_scalar(sl, ps_o[:], coef[:], None, op0=op.mult)
                    nc.sync.dma_start(
                        o_d[n * 128:(n + 1) * 128, oh * 512:(oh + 1) * 512], sl
                    )

            # ---------------- weight phase ---------------------------------
            with tc.tile_pool(name="wpool", bufs=1) as wpool, \
                 tc.tile_pool(name="wtmp", bufs=3) as wtmp:
                w_t = [
                    wpool.tile([128, D_IN], F32, name=f"w{ob}", tag=f"w{ob}")
                    for ob in range(OB)
                ]
                # w first on both queues, then x tiles behind
                for ob in range(4):
                    nc.sync.dma_start(w_t[ob][:], w_d[ob * 128:(ob + 1) * 128, :])
                for ob in range(4, OB):
                    nc.scalar.dma_start(w_t[ob][:], w_d[ob * 128:(ob + 1) * 128, :])
                a_disp(0, nc.sync)
                a_disp(1, nc.scalar)
                a_disp(2, nc.sync)
                a_disp(3, nc.sync)

                masks.make_identity(nc, ident_f32[:])
                masks.make_identity(nc, ident_bf[:])
                nc.vector.memset(ones_col[:], 1.0)
                nc.vector.memset(ones_row[:], 1.0)

                # raw w transposed, fp32: wT_all[:, k, o] = w[o, k*128 + p]
                wT_all = wpool.tile([128, KT, D_OUT], F32, name="wT_all")
                colsum = wpool.tile([128, OB], F32, name="colsum")
                for ob in range(OB):
                    nc.vector.tensor_reduce(
                        colsum[:, ob:ob + 1], w_t[ob][:], axis=AX, op=op.add,
                        apply_absolute_value=True,
                    )
                    for h in range(2):
                        po_w = psu.tile([128, KT // 2, 128], F32, tag="ps", name="po_w")
                        for j in range(KT // 2):
                            k = h * (KT // 2) + j
                            nc.tensor.transpose(
                                po_w[:, j, :],
                                w_t[ob][:, k * 128:(k + 1) * 128],
                                ident_f32[:],
                            )
                        nc.scalar.activation(
                            wT_all[:, h * (KT // 2):(h + 1) * (KT // 2),
                                   ob * 128:(ob + 1) * 128],
                            po_w[:], Copy,
                        )

                # global scale s = clip(mean|W|, EPS); bc2 = [s, 1/s] broadcast
                colsum2 = wpool.tile([128, 1], F32, name="colsum2")
                nc.vector.tensor_reduce(colsum2[:], colsum[:], axis=AX, op=op.add)
                ps_st = psu.tile([128, 512], F32, tag="ps", name="ps_st")
                nc.tensor.matmul(ps_st[0:1, 0:1], ones_col[:], colsum2[:])
                pair = wpool.tile([1, 2], F32, name="pair")
                nc.scalar.activation(
                    pair[:, 0:1], ps_st[0:1, 0:1], Copy, scale=1.0 / (D_OUT * D_IN)
                )
                nc.vector.tensor_scalar_max(pair[:, 0:1], pair[:, 0:1], EPS)
                nc.vector.reciprocal(pair[:, 1:2], pair[:, 0:1])
                ps_st2 = psu.tile([128, 512], F32, tag="ps", name="ps_st2")
                nc.tensor.matmul(ps_st2[:, 0:2], ones_row[:], pair[:])
                nc.scalar.copy(bc2[:], ps_st2[:, 0:2])
                nc.vector.tensor_scalar(
                    s127_bc[:], bc2[:, 0:1], 1.0 / QMAX, None, op0=op.mult
                )

                # ternarize per k (magic domain) + interleaved b0/b1 chains +
                # x0..x3 prep, all mixed so no engine serializes the window
                ch = [
                    [
                        psu.tile([128, 512], F32, tag="ps", name=f"ch{i}_{oh}")
                        for oh in range(2)
                    ]
                    for i in range(2)
                ]
                ACT_UNMAGIC = {3, 6}
                GP_UNMAGIC = set()   # gpsimd measured ~15us per [128,1024] pass — useless
                for it in range(KT + 3):
                    if it < KT:
                        k = it
                        y0 = wtmp.tile([128, D_OUT], F32, name="y0", tag="y0")
                        nc.scalar.activation(
                            y0[:], wT_all[:, k, :], Copy, bias=MAGIC,
                            scale=bc2[:, 1:2],
                        )
                        y1 = wtmp.tile([128, D_OUT], F32, name="y1", tag="y1")
                        nc.vector.tensor_scalar(
                            y1[:], y0[:], MAGIC + 1.0, MAGIC - 1.0,
                            op0=op.min, op1=op.max,
                        )
                        if k in ACT_UNMAGIC:
                            nc.scalar.activation(tT[k][:], y1[:], Copy, bias=-MAGIC)
                        elif k in GP_UNMAGIC:
                            nc.gpsimd.tensor_scalar(
                                tT[k][:], y1[:], -MAGIC, None, op0=op.add
                            )
                        else:
                            nc.vector.tensor_scalar(
                                tT[k][:], y1[:], -MAGIC, None, op0=op.add
                            )
                    if it == 0:
                        a_stats(0)
                        a_round(0, dve=True)
                    elif it == 1:
                        a_xpose(0)
                        a_stats(1)
                        a_round(1, dve=True)
                    elif it == 2:
                        a_xpose(1)
                    if 2 <= it < 2 + KT:
                        b_mm(ch[0], live[("qT", 0)], it - 2)
                    if 3 <= it < 3 + KT:
                        b_mm(ch[1], live[("qT", 1)], it - 3)
                    if it == 3:
                        a_disp(4, nc.sync)
                    elif it == 4:
                        a_stats(2)
                        a_round(2)
                    elif it == 5:
                        a_xpose(2)
                        a_disp(5, nc.sync)
                    elif it == 6:
                        a_stats(3)
                        a_round(3)
                    elif it == 7:
                        a_xpose(3)
                        a_disp(6, nc.sync)
                    elif it == 8:
                        a_disp(7, nc.sync)
                        a_stats(4)
                        a_round(4)
                    elif it == 9:
                        a_xpose(4)
                live.pop(("qT", 0))
                live.pop(("qT", 1))

            b_drain(0, ch[0])
            b_drain(1, ch[1])

            # ---------------- steady state ---------------------------------
            # quant lookahead of 3 tiles: qr(m) is produced a full tile before
            # the PE reaches its transposes, so DMA/engine jitter never lands
            # on the PE's critical path
            for n in range(2, TILES):
                b(n)
                if n + 6 < TILES:
                    a_disp(n + 6, nc.sync)
                m = n + 3
                if 5 <= m < TILES:
                    a_stats(m)
                    a_round(m)
                    a_xpose(m)

    _split_multiwaits(nc)
    return nc


_NC_CACHE = None


def _get_nc():
    global _NC_CACHE
    if _NC_CACHE is None:
        _NC_CACHE = build_program()
    return _NC_CACHE


def kernel(x: np.ndarray, weight: np.ndarray, trace: bool = False):
    assert x.shape == (B, S, D_IN) and weight.shape == (D_OUT, D_IN)
    nc = _get_nc()
    xf = np.ascontiguousarray(x.reshape(TOKENS, D_IN), dtype=np.float32)
    w = np.ascontiguousarray(weight, dtype=np.float32)
    in_maps = [
        {
            "x": xf[c * TOK_PER_CORE:(c + 1) * TOK_PER_CORE],
            "weight": w,
        }
        for c in range(N_CORES)
    ]
    res = run_bass_kernel_spmd(nc, in_maps, core_ids=list(range(N_CORES)), trace=trace)
    kernel.last_results = res
    out = np.concatenate([res.results[c]["out"] for c in range(N_CORES)], axis=0)
    return out.reshape(B, S, D_OUT)


kernel.last_results = None
